# revision 1
# baseline (speedup 1.0000x reference)
"""Trainium2 Bass kernel for nn_CIFARDiffusionLayer.

The reference applies, per channel c, three ADI steps; each step is an
x-sweep (constant-coefficient tridiagonal solve along W), a y-sweep
(same along H), and a multiply by diag(channel_coupling)[c].  Every
sweep is a fixed linear map: solving T x = d with the reference's exact
Thomas recurrence is x = T^{-1} d, and T^{-1} is a dense 256x256 matrix
that depends only on (channel, step, direction).  X-sweeps act on U by
right-multiplication and y-sweeps by left-multiplication, so they all
commute across steps and the whole layer collapses to

    out[b, c] = A_c @ u[b, c] @ B_c
    A_c = s_c^3 * My(c,2) @ My(c,1) @ My(c,0)      (s_c = coupling diag)
    B_c = Mx(c,0)^T @ Mx(c,1)^T @ Mx(c,2)^T

with the tiny 256x256 matrices computed on the host in float64 from the
reference's exact recurrences (including its eps quirks).  The device
work is two 256x256x256 matmuls per (batch, channel) slab, run as
fp32r (full-rate) TensorE matmuls with the data slab as the stationary
operand so each matmul also transposes the slab back and forth.

Sharding: pure data parallelism over batch B=128 -> 16 batches per core
across 8 cores; the matrices are replicated.
"""

import sys

if "/opt/trn_rl_repo" not in sys.path:
    sys.path.insert(0, "/opt/trn_rl_repo")

import numpy as np

DT = 0.05
DX = 1.0
NUM_STEPS = 3
EPS = 1e-6
MAX_COEFF = 1.0

N_CORES = 8
B, C, S = 128, 3, 256
B_LOC = B // N_CORES


def _thomas_inv(r: float, n: int = S, eps: float = EPS) -> np.ndarray:
    """T^{-1} for the reference's constant-coefficient Thomas solve.

    Mirrors reference._thomas_const exactly (b[0]+eps on the first
    denominator, clamp(min=eps) on interior denominators), evaluated in
    float64 on the identity RHS so columns are T^{-1} e_j.
    """
    a = -r
    b = np.full(n, 1.0 + 2.0 * r, dtype=np.float64)
    b[0] = b[-1] = 1.0 + r
    denom = np.empty(n, dtype=np.float64)
    cp = np.empty(n, dtype=np.float64)
    denom[0] = b[0] + eps
    cp[0] = a / denom[0]
    for i in range(1, n):
        denom[i] = max(b[i] - a * cp[i - 1], eps)
        cp[i] = a / denom[i]
    dp = np.zeros((n, n), dtype=np.float64)
    eye = np.eye(n, dtype=np.float64)
    dp[0] = eye[0] / denom[0]
    for i in range(1, n):
        dp[i] = (eye[i] - a * dp[i - 1]) / denom[i]
    x = np.zeros((n, n), dtype=np.float64)
    x[n - 1] = dp[n - 1]
    for i in range(n - 2, -1, -1):
        x[i] = dp[i] - cp[i] * x[i + 1]
    return x


def _host_mats(alpha_base, beta_base, alpha_spatial, beta_spatial, channel_coupling):
    """mats[c, 0] = A_c^T, mats[c, 1] = B_c, as float32 [C, 2, S, S]."""
    diag = np.diagonal(np.asarray(channel_coupling)).astype(np.float64)
    mats = np.empty((C, 2, S, S), dtype=np.float32)
    for c in range(C):
        am = float(np.mean(np.asarray(alpha_spatial[c], dtype=np.float64)))
        bm = float(np.mean(np.asarray(beta_spatial[c], dtype=np.float64)))
        a_c = np.eye(S, dtype=np.float64)
        b_c = np.eye(S, dtype=np.float64)
        for step in range(NUM_STEPS):
            t = step * DT
            alpha_t = min(max(float(alpha_base[c]) + am * t, EPS), MAX_COEFF)
            beta_t = min(max(float(beta_base[c]) + bm * t, EPS), MAX_COEFF)
            r_a = alpha_t * (DT / 2.0) / DX**2
            r_b = beta_t * (DT / 2.0) / DX**2
            a_c = _thomas_inv(r_b) @ a_c
            b_c = b_c @ _thomas_inv(r_a).T
        mats[c, 0] = (diag[c] ** 3 * a_c).T.astype(np.float32)
        mats[c, 1] = b_c.astype(np.float32)
    return mats


def build_module():
    """Per-core Bass module: out[b,c] = A_c @ u[b,c] @ B_c for 16 slabs x 3 ch."""
    import concourse.bacc as bacc
    import concourse.tile as tile
    from concourse import mybir

    f32, f32r = mybir.dt.float32, mybir.dt.float32r
    nc = bacc.Bacc(
        "TRN2",
        target_bir_lowering=False,
        debug=False,
        enable_asserts=False,
        num_devices=N_CORES,
    )
    u_d = nc.dram_tensor("u", [B_LOC, C, S, S], f32r, kind="ExternalInput")
    m_d = nc.dram_tensor("mats", [C, 2, S, S], f32r, kind="ExternalInput")
    o_d = nc.dram_tensor("out", [B_LOC, C, S, S], f32, kind="ExternalOutput")

    with tile.TileContext(nc) as tc:
        with (
            tc.tile_pool(name="consts", bufs=1) as cpool,
            tc.tile_pool(name="ld", bufs=4) as ldpool,
            tc.tile_pool(name="vt", bufs=3) as vtpool,
            tc.tile_pool(name="zs", bufs=3) as zspool,
            tc.tile_pool(name="pv", bufs=2, space="PSUM") as pvpool,
            tc.tile_pool(name="pz", bufs=2, space="PSUM") as pzpool,
        ):
            # Constant matrices, one [128, 512] tile per (channel, side):
            # [:, 0:256] = k-tile rows 0..127, [:, 256:512] = rows 128..255.
            a_t, b_t = [], []
            for c in range(C):
                at = cpool.tile([128, 512], f32r, tag=f"a{c}")
                nc.sync.dma_start(at[:], m_d[c, 0].rearrange("(k p) w -> p k w", p=128))
                a_t.append(at)
                bt = cpool.tile([128, 512], f32r, tag=f"b{c}")
                nc.sync.dma_start(bt[:], m_d[c, 1].rearrange("(k p) w -> p k w", p=128))
                b_t.append(bt)

            for b in range(B_LOC):
                # One 768KB load: free layout c*512 + k*256 + w, partition = h%128.
                ld = ldpool.tile([128, C * 512], f32r)
                nc.sync.dma_start(ld[:], u_d[b].rearrange("c (k p) w -> p c k w", p=128))
                zs = zspool.tile([128, C * 512], f32)
                for c in range(C):
                    base = c * 512
                    # MM1: V^T[w, h'] = sum_h U[h, w] * A^T[h, h']  (data stationary)
                    pv = pvpool.tile([128, 512], f32)
                    for mi in range(2):
                        for k in range(2):
                            nc.tensor.matmul(
                                pv[:, mi * 256 : (mi + 1) * 256],
                                ld[:, base + k * 256 + mi * 128 : base + k * 256 + mi * 128 + 128],
                                a_t[c][:, k * 256 : (k + 1) * 256],
                                start=(k == 0),
                                stop=(k == 1),
                            )
                    vt = vtpool.tile([128, 512], f32r)
                    nc.vector.tensor_copy(vt[:], pv[:])
                    # MM2: Z[h', w'] = sum_w V^T[w, h'] * B[w, w']
                    pz = pzpool.tile([128, 512], f32)
                    for mi in range(2):
                        for k in range(2):
                            nc.tensor.matmul(
                                pz[:, mi * 256 : (mi + 1) * 256],
                                vt[:, k * 256 + mi * 128 : k * 256 + mi * 128 + 128],
                                b_t[c][:, k * 256 : (k + 1) * 256],
                                start=(k == 0),
                                stop=(k == 1),
                            )
                    nc.scalar.copy(zs[:, base : base + 512], pz[:])
                nc.sync.dma_start(o_d[b].rearrange("c (k p) w -> p c k w", p=128), zs[:])
    nc.compile()
    return nc


def kernel(u, alpha_base, beta_base, alpha_spatial, beta_spatial, channel_coupling):
    from concourse.bass_utils import run_bass_kernel_spmd

    u = np.ascontiguousarray(np.asarray(u, dtype=np.float32))
    mats = _host_mats(
        np.asarray(alpha_base, dtype=np.float32),
        np.asarray(beta_base, dtype=np.float32),
        np.asarray(alpha_spatial, dtype=np.float32),
        np.asarray(beta_spatial, dtype=np.float32),
        np.asarray(channel_coupling, dtype=np.float32),
    )
    nc = build_module()
    in_maps = [
        {"u": u[i * B_LOC : (i + 1) * B_LOC], "mats": mats} for i in range(N_CORES)
    ]
    res = run_bass_kernel_spmd(nc, in_maps, core_ids=list(range(N_CORES)))
    return np.concatenate([r["out"] for r in res.results], axis=0)


# revision 2
# speedup vs baseline: 1051.3070x; 1051.3070x over previous
"""Trainium2 Bass kernel for nn_CIFARDiffusionLayer.

The reference applies, per channel c, three ADI steps; each step is an
x-sweep (constant-coefficient tridiagonal solve along W), a y-sweep
(same along H), and a multiply by diag(channel_coupling)[c].  Every
sweep is a fixed linear map: solving T x = d with the reference's exact
Thomas recurrence is x = T^{-1} d, and T^{-1} is a dense 256x256 matrix
that depends only on (channel, step, direction).  X-sweeps act on U by
right-multiplication and y-sweeps by left-multiplication, so they all
commute across steps and the whole layer collapses to

    out[b, c] = A_c @ u[b, c] @ B_c
    A_c = s_c^3 * My(c,2) @ My(c,1) @ My(c,0)      (s_c = coupling diag)
    B_c = Mx(c,0)^T @ Mx(c,1)^T @ Mx(c,2)^T

with the tiny 256x256 matrices computed on the host in float64 from the
reference's exact recurrences (including its eps quirks).  The device
work is two 256x256x256 matmuls per (batch, channel) slab, run as
fp32r (full-rate) TensorE matmuls with the data slab as the stationary
operand so each matmul also transposes the slab back and forth.

Sharding: pure data parallelism over batch B=128 -> 16 batches per core
across 8 cores; the matrices are replicated.
"""

import sys

if "/opt/trn_rl_repo" not in sys.path:
    sys.path.insert(0, "/opt/trn_rl_repo")

import numpy as np

DT = 0.05
DX = 1.0
NUM_STEPS = 3
EPS = 1e-6
MAX_COEFF = 1.0

N_CORES = 8
B, C, S = 128, 3, 256
B_LOC = B // N_CORES


def _thomas_inv(r: float, n: int = S, eps: float = EPS) -> np.ndarray:
    """T^{-1} for the reference's constant-coefficient Thomas solve.

    Mirrors reference._thomas_const exactly (b[0]+eps on the first
    denominator, clamp(min=eps) on interior denominators), evaluated in
    float64 on the identity RHS so columns are T^{-1} e_j.
    """
    a = -r
    b = np.full(n, 1.0 + 2.0 * r, dtype=np.float64)
    b[0] = b[-1] = 1.0 + r
    denom = np.empty(n, dtype=np.float64)
    cp = np.empty(n, dtype=np.float64)
    denom[0] = b[0] + eps
    cp[0] = a / denom[0]
    for i in range(1, n):
        denom[i] = max(b[i] - a * cp[i - 1], eps)
        cp[i] = a / denom[i]
    dp = np.zeros((n, n), dtype=np.float64)
    eye = np.eye(n, dtype=np.float64)
    dp[0] = eye[0] / denom[0]
    for i in range(1, n):
        dp[i] = (eye[i] - a * dp[i - 1]) / denom[i]
    x = np.zeros((n, n), dtype=np.float64)
    x[n - 1] = dp[n - 1]
    for i in range(n - 2, -1, -1):
        x[i] = dp[i] - cp[i] * x[i + 1]
    return x


def _host_mats(alpha_base, beta_base, alpha_spatial, beta_spatial, channel_coupling):
    """mats[c, 0] = A_c^T, mats[c, 1] = B_c, as float32 [C, 2, S, S]."""
    diag = np.diagonal(np.asarray(channel_coupling)).astype(np.float64)
    mats = np.empty((C, 2, S, S), dtype=np.float32)
    for c in range(C):
        am = float(np.mean(np.asarray(alpha_spatial[c], dtype=np.float64)))
        bm = float(np.mean(np.asarray(beta_spatial[c], dtype=np.float64)))
        a_c = np.eye(S, dtype=np.float64)
        b_c = np.eye(S, dtype=np.float64)
        for step in range(NUM_STEPS):
            t = step * DT
            alpha_t = min(max(float(alpha_base[c]) + am * t, EPS), MAX_COEFF)
            beta_t = min(max(float(beta_base[c]) + bm * t, EPS), MAX_COEFF)
            r_a = alpha_t * (DT / 2.0) / DX**2
            r_b = beta_t * (DT / 2.0) / DX**2
            a_c = _thomas_inv(r_b) @ a_c
            b_c = b_c @ _thomas_inv(r_a).T
        mats[c, 0] = (diag[c] ** 3 * a_c).T.astype(np.float32)
        mats[c, 1] = b_c.astype(np.float32)
    return mats


def build_module():
    """Per-core Bass module: out[b,c] = A_c @ u[b,c] @ B_c for 16 slabs x 3 ch."""
    import concourse.bacc as bacc
    import concourse.tile as tile
    from concourse import mybir

    f32, f32r = mybir.dt.float32, mybir.dt.float32r
    nc = bacc.Bacc(
        "TRN2",
        target_bir_lowering=False,
        debug=False,
        enable_asserts=False,
        num_devices=N_CORES,
    )
    u_d = nc.dram_tensor("u", [B_LOC, C, S, S], f32r, kind="ExternalInput")
    m_d = nc.dram_tensor("mats", [C, 2, S, S], f32r, kind="ExternalInput")
    o_d = nc.dram_tensor("out", [B_LOC, C, S, S], f32, kind="ExternalOutput")

    with tile.TileContext(nc) as tc:
        with (
            tc.tile_pool(name="consts", bufs=1) as cpool,
            tc.tile_pool(name="ld", bufs=4) as ldpool,
            tc.tile_pool(name="vt", bufs=3) as vtpool,
            tc.tile_pool(name="zs", bufs=4) as zspool,
            tc.tile_pool(name="pv", bufs=2, space="PSUM") as pvpool,
            tc.tile_pool(name="pz", bufs=2, space="PSUM") as pzpool,
        ):
            # Constant matrices, one [128, 512] tile per (channel, side):
            # [:, 0:256] = k-tile rows 0..127, [:, 256:512] = rows 128..255.
            a_t, b_t = [], []
            for c in range(C):
                at = cpool.tile([128, 512], f32r, tag=f"a{c}")
                nc.sync.dma_start(at[:], m_d[c, 0].rearrange("(k p) w -> p k w", p=128))
                a_t.append(at)
                bt = cpool.tile([128, 512], f32r, tag=f"b{c}")
                nc.sync.dma_start(bt[:], m_d[c, 1].rearrange("(k p) w -> p k w", p=128))
                b_t.append(bt)

            for b in range(B_LOC):
                # Load u[b]: free layout c*512 + k*256 + w, partition = h%128.
                # Per-channel DMAs keep the SP queue from head-of-line
                # blocking and give the scheduler finer overlap granularity.
                ld = ldpool.tile([128, C * 512], f32r)
                for c in range(C):
                    nc.sync.dma_start(
                        ld[:, c * 512 : (c + 1) * 512],
                        u_d[b, c].rearrange("(k p) w -> p k w", p=128),
                    )
                zs = zspool.tile([128, C * 512], f32)
                for c in range(C):
                    base = c * 512
                    # MM1: V^T[w, h'] = sum_h U[h, w] * A^T[h, h']  (data stationary)
                    pv = pvpool.tile([128, 512], f32)
                    for mi in range(2):
                        for k in range(2):
                            nc.tensor.matmul(
                                pv[:, mi * 256 : (mi + 1) * 256],
                                ld[:, base + k * 256 + mi * 128 : base + k * 256 + mi * 128 + 128],
                                a_t[c][:, k * 256 : (k + 1) * 256],
                                start=(k == 0),
                                stop=(k == 1),
                            )
                    vt = vtpool.tile([128, 512], f32r)
                    nc.vector.tensor_copy(vt[:], pv[:])
                    # MM2: Z[h', w'] = sum_w V^T[w, h'] * B[w, w']
                    pz = pzpool.tile([128, 512], f32)
                    for mi in range(2):
                        for k in range(2):
                            nc.tensor.matmul(
                                pz[:, mi * 256 : (mi + 1) * 256],
                                vt[:, k * 256 + mi * 128 : k * 256 + mi * 128 + 128],
                                b_t[c][:, k * 256 : (k + 1) * 256],
                                start=(k == 0),
                                stop=(k == 1),
                            )
                    nc.scalar.copy(zs[:, base : base + 512], pz[:])
                # Out-DMA on the ACT HWDGE ring: keeps the SP queue free for
                # input loads (out-DMAs wait on compute; SP head-of-line
                # blocking would stall the next batch's loads behind them).
                nc.scalar.dma_start(o_d[b].rearrange("c (k p) w -> p c k w", p=128), zs[:])
    nc.compile()
    return nc


def kernel(u, alpha_base, beta_base, alpha_spatial, beta_spatial, channel_coupling):
    from concourse.bass_utils import run_bass_kernel_spmd

    u = np.ascontiguousarray(np.asarray(u, dtype=np.float32))
    mats = _host_mats(
        np.asarray(alpha_base, dtype=np.float32),
        np.asarray(beta_base, dtype=np.float32),
        np.asarray(alpha_spatial, dtype=np.float32),
        np.asarray(beta_spatial, dtype=np.float32),
        np.asarray(channel_coupling, dtype=np.float32),
    )
    nc = build_module()
    in_maps = [
        {"u": u[i * B_LOC : (i + 1) * B_LOC], "mats": mats} for i in range(N_CORES)
    ]
    res = run_bass_kernel_spmd(nc, in_maps, core_ids=list(range(N_CORES)))
    return np.concatenate([r["out"] for r in res.results], axis=0)
